# revision 19
# baseline (speedup 1.0000x reference)
"""Causal self-attention Trainium2 kernel (B=2, T=2048, C=1024, H=16).

Sharding: tensor-parallel over heads (4-way) x data-parallel over batch (2-way)
= 8 cores. Core c handles batch b = c//4 and heads [4*(c%4), 4*(c%4)+4).
Each core computes x @ W_attn for its head slice, causal attention for its 4
heads, and a partial y @ W_proj over its 256 channels. The host sums the 4
partials per batch element (no device collectives).

Matmul operands are fp16 (full-rate PE; fp32 matmul is 4x slower). All PSUM
accumulation is fp32. Weights are host-cast to fp16.

Layouts (per core, b fixed):
  xT   [c, t]    : 8 c-tiles of [128, 2048]  (DMA-xbar transposed from x)
  qT/kT[d', t]   : per head-pair hp, [128, 2048]; partitions 0-63 = head 2hp,
                   64-127 = head 2hp+1
  vp   [s, h, d']: [128, 16 s-tiles, 4 heads, 65]; col 64 is a ones-column so
                   PV emits the softmax denominator for free
  sp   [s, hi, t]: scores for one s-tile, both heads of pair hp, PSUM
                   [128,2,512]; one exp covers both heads, causally trimmed
  y2   [t, h, 65]: PV output t-major in a 3-bank PSUM arena; col 64 is the
                   denominator. Normalize is a per-partition tensor_scalar
                   multiply, then a PE transpose back to [c', t] for proj.

Emission is si-outer: per s-tile, QK -> exp -> PV matmuls for every t-tile
at or above the diagonal, so the PE has PV work during the exp-bound early
s-tiles and the per-block tail is tiny. qkv matmuls for later t-blocks are
drip-fed into the stream with per-unit deadlines.
"""

import sys

if "/opt/trn_rl_repo" not in sys.path:
    sys.path.insert(0, "/opt/trn_rl_repo")

import numpy as np

import concourse.bass as bass
import concourse.bacc as bacc
import concourse.mybir as mybir
import concourse.tile as tile
from concourse.bass_utils import run_bass_kernel_spmd

F32 = mybir.dt.float32
F16 = mybir.dt.float16

B, T, C = 2, 2048, 1024
NH = 16              # total heads
D = 64               # head dim
N_CORES = 8
HG = 4               # heads per core
FC = HG * D          # 256 f-columns per core per q/k/v
CT = C // 128        # 8 c-tiles
TT = T // 128        # 16 t-tiles / s-tiles
TB = T // 512        # 4 t-blocks
SCALE = 1.0 / 8.0    # 1/sqrt(D)


def build():
    nc = bacc.Bacc("TRN2", target_bir_lowering=False, debug=False,
                   num_devices=N_CORES)
    x_d = nc.dram_tensor("x", [T, C], F16, kind="ExternalInput").ap()
    wq_d = nc.dram_tensor("wq", [C, FC], F16, kind="ExternalInput").ap()
    wk_d = nc.dram_tensor("wk", [C, FC], F16, kind="ExternalInput").ap()
    wv_d = nc.dram_tensor("wv", [C, FC], F16, kind="ExternalInput").ap()
    wp_d = nc.dram_tensor("wp", [FC, C], F16, kind="ExternalInput").ap()
    out_d = nc.dram_tensor("out", [T, C], F32, kind="ExternalOutput").ap()

    with tile.TileContext(nc) as tc:
        body(tc, x_d, wq_d, wk_d, wv_d, wp_d, out_d)
    nc.compile()
    return nc


def body(tc, x_d, wq_d, wk_d, wv_d, wp_d, out_d):
    nc = tc.nc
    Exp = mybir.ActivationFunctionType.Exp

    with (
        tc.tile_pool(name="sb", bufs=1) as sb,
        tc.tile_pool(name="ps", bufs=1, space="PSUM") as ps,
    ):
        # binary causal mask in S^T orientation: 1 where t - s >= 0 else 0
        mask = sb.tile([128, 128], F16)
        nc.gpsimd.memset(mask, 1.0)
        nc.gpsimd.affine_select(
            out=mask, in_=mask, compare_op=mybir.AluOpType.is_ge,
            fill=0.0, base=0, pattern=[[1, 128]], channel_multiplier=-1)
        # identity for PE transposes: intersect the two triangle selects
        ident = sb.tile([128, 128], F16)
        nc.gpsimd.memset(ident, 1.0)
        nc.gpsimd.affine_select(
            out=ident, in_=ident, compare_op=mybir.AluOpType.is_ge,
            fill=0.0, base=0, pattern=[[1, 128]], channel_multiplier=-1)
        nc.gpsimd.affine_select(
            out=ident, in_=ident, compare_op=mybir.AluOpType.is_ge,
            fill=0.0, base=0, pattern=[[-1, 128]], channel_multiplier=1)
        ones = sb.tile([128, 64], F16)
        nc.gpsimd.memset(ones, 1.0)

        wq_sb = sb.tile([128, CT, FC], F16)
        wk_sb = sb.tile([128, CT, FC], F16)
        wv_sb = sb.tile([128, CT, FC], F16)
        wp_sb = sb.tile([128, 2, C], F16)          # [c'(128), hp, n]
        nc.gpsimd.dma_start(wv_sb, wv_d.rearrange("(ct p) f -> p ct f", p=128))
        xT = sb.tile([128, CT, T], F16)            # [c_local, ct, t]
        qT = sb.tile([128, 2, T], F16)             # [d', hp, t]
        kT = sb.tile([128, 2, T], F16)
        vp = sb.tile([128, TT, HG, 65], F16)       # [s_in_tile, s_tile, h, d'+1]
        pt0 = sb.tile([128, TT, 2, 512], F16)      # [s, s_tile, hi, t_in_tb]
        pt1 = sb.tile([128, TT, 2, 512], F16)
        pts = (pt0, pt1)
        yT = sb.tile([128, 2, T], F16)             # [c', hp, t]
        nc.gpsimd.memset(vp[:, :, :, 64:65], 1.0)  # PV denominator column

        # transpose x into xT with the DMA xbar (fp16, 2-byte dtype),
        # one [512, 128] -> [128, 512] transpose per (t-block, c-tile).
        # weight loads go out on the Pool engine's DMA queue so SP's slow
        # per-instruction issue (~860ns) is spent only on the x transposes,
        # which gate the compute start
        nc.gpsimd.dma_start(wq_sb, wq_d.rearrange("(ct p) f -> p ct f", p=128))
        nc.gpsimd.dma_start(wk_sb, wk_d.rearrange("(ct p) f -> p ct f", p=128))
        nc.gpsimd.dma_start(wp_sb, wp_d.rearrange("(hp p) n -> p hp n", p=128))
        for ci in range(CT):
            nc.sync.dma_start(
                xT[:, ci, :], x_d[:, ci * 128:(ci + 1) * 128],
                transpose=True)

        # PSUM: sp 2x2 banks + qv 1 + arena 3 = 8 banks
        def sp_tile(name):
            return ps.tile([128, 2, 512], F32, tag="sp", name=name, bufs=2)

        def qv_tile(name):
            return ps.tile([128, 512], F32, tag="qv", name=name, bufs=1)

        # 3-bank arena: 16 PV accumulation regions of 65 fp32 columns
        # (region r = ttl*4+h at bank r//7, slot r%7). PSUM start_tensor_calc
        # zeroes a whole 2KB bank, which would wipe sibling regions, so the
        # arena is zeroed by DVE memset per t-block and every PV matmul
        # accumulates with start=False.
        arena = ps.tile([128, 1536], F32, name="arena")

        def reg(h, ttl):
            r = ttl * 4 + h
            bank, idx = divmod(r, 7)
            c0 = bank * 512 + idx * 65
            return arena[:, c0:c0 + 65]

        # HAM warmup: the PE is DMA-blocked at startup; dependency-free
        # matmuls on constants un-throttle the clock, and one tiny exp
        # prefetches the spline table.
        for _ in range(16):
            warm = qv_tile("warm")
            nc.tensor.matmul(warm[0:64, 0:128], lhsT=ones, rhs=mask,
                             start=True, stop=True)
            nc.tensor.matmul(warm[0:64, 128:256], lhsT=ones, rhs=mask,
                             start=True, stop=True)
        warm_e = sb.tile([1, 64], F16)
        nc.scalar.activation(warm_e, ones[0:1, :], Exp, scale=SCALE)

        def emit_v(tt):
            v_ps = qv_tile("v_ps")
            for ci in range(CT):
                nc.tensor.matmul(
                    v_ps[:, 0:256],
                    lhsT=xT[:, ci, tt * 128:(tt + 1) * 128],
                    rhs=wv_sb[:, ci, :],
                    start=(ci == 0), stop=(ci == CT - 1))
            nc.vector.tensor_copy(
                vp[:, tt, :, 0:64],
                v_ps[:, 0:256].rearrange("p (h d) -> p h d", h=HG))

        def emit_qk(hp, w_sb, dst, tb):
            qk_ps = qv_tile("qk_ps")
            for ci in range(CT):
                nc.tensor.matmul(
                    qk_ps,
                    lhsT=w_sb[:, ci, hp * 128:(hp + 1) * 128],
                    rhs=xT[:, ci, tb * 512:(tb + 1) * 512],
                    start=(ci == 0), stop=(ci == CT - 1))
            nc.vector.tensor_copy(
                dst[:, hp, tb * 512:(tb + 1) * 512], qk_ps)

        def emit_qk_si(hp, tb, si):
            # scores S^T for one s-tile, both heads of pair hp, then one
            # exp covering both heads with the causal prefix trimmed
            kd = si - 4 * tb
            col0 = 128 * kd if kd > 0 else 0
            sp = sp_tile("sp")
            for hi in (0, 1):
                nc.tensor.matmul(
                    sp[:, hi, col0:512],
                    lhsT=kT[64 * hi:64 * hi + 64, hp,
                            si * 128:(si + 1) * 128],
                    rhs=qT[64 * hi:64 * hi + 64, hp,
                           tb * 512 + col0:(tb + 1) * 512],
                    start=True, stop=True)
            pt = pts[hp]
            nc.scalar.activation(pt[:, si, :, col0:512], sp[:, :, col0:512],
                                 Exp, scale=SCALE)
            if kd >= 0:
                # zero the invalid triangle of the diagonal square after
                # exp (exp * 0 == masked exp, off the S -> exp hot path)
                for hi in (0, 1):
                    psl = pt[:, si, hi, col0:col0 + 128]
                    nc.vector.tensor_mul(psl, psl, mask)

        def emit_pv_si(tb, si):
            # PV matmuls of s-tile si into every t-tile at/above the
            # diagonal; chain (h, ttl) accumulates over si and closes at
            # the diagonal
            for ttl in range(max(0, si - 4 * tb), 4):
                tt = 4 * tb + ttl
                for h in range(HG):
                    hp, hi = h // 2, h % 2
                    nc.tensor.matmul(
                        reg(h, ttl),
                        lhsT=pts[hp][:, si, hi, ttl * 128:(ttl + 1) * 128],
                        rhs=vp[:, si, h, :],
                        start=False, stop=(si == tt),
                        skip_group_check=True)

        def region_runs(ttl):
            """maximal same-bank runs of the 4 regions of t-tile ttl"""
            runs = []
            h0 = 0
            while h0 < HG:
                r0 = ttl * 4 + h0
                n = min(HG - h0, 7 - r0 % 7)
                bank, idx = divmod(r0, 7)
                runs.append((h0, n, bank * 512 + idx * 65))
                h0 += n
            return runs

        def emit_norm(tb, tt):
            ttl = tt - 4 * tb
            runs = region_runs(ttl)
            rcp = sb.tile([128, HG], F32, tag="rcp", name="rcp", bufs=3)
            for h0, n, base in runs:
                sl = arena[:, base:base + 65 * n].rearrange(
                    "p (n x) -> p n x", n=n)
                nc.vector.reciprocal(rcp[:, h0:h0 + n], sl[:, :, 64])
            y_sb = sb.tile([128, FC], F16, tag="ysb", name="y_sb", bufs=3)
            for h0, n, base in runs:
                sl = arena[:, base:base + 65 * n].rearrange(
                    "p (n x) -> p n x", n=n)
                nc.vector.tensor_mul(
                    y_sb[:, h0 * 64:(h0 + n) * 64].rearrange(
                        "p (n x) -> p n x", n=n),
                    sl[:, :, 0:64],
                    rcp[:, h0:h0 + n].unsqueeze(2).broadcast_to([128, n, 64]))
            # PE transpose [t, c'] -> [c', t] for the projection lhsT; the
            # scratch borrows an sp rotation slot (start=True zeroing is
            # safe there, unlike in the accumulating arena)
            yt = sp_tile("yt")[:, 0, 0:128].bitcast(F16)   # [128, 256] f16
            for ch in range(2):
                nc.tensor.transpose(
                    yt[:, ch * 128:(ch + 1) * 128],
                    y_sb[:, ch * 128:(ch + 1) * 128], ident)
            nc.vector.tensor_copy(
                yT[:, :, tt * 128:(tt + 1) * 128],
                yt.rearrange("p (hp t) -> p hp t", hp=2))
            # re-zero this t-tile's regions for the next block's start=False
            # accumulation chains (WAR-ordered after the reads above)
            for h0, n, base in region_runs(ttl):
                nc.vector.memset(arena[:, base:base + 65 * n], 0.0)

        def emit_proj(tt):
            pj = sp_tile("pj")
            for nb in range(2):
                for hp in range(2):
                    nc.tensor.matmul(
                        pj[:, nb, :],
                        lhsT=yT[:, hp, tt * 128:(tt + 1) * 128],
                        rhs=wp_sb[:, hp, nb * 512:(nb + 1) * 512],
                        start=(hp == 0), stop=(hp == 1))
            ob = sb.tile([128, C], F32, tag="ob", name="ob", bufs=3)
            nc.scalar.copy(ob[:, 0:512], pj[:, 0, :])
            nc.vector.tensor_copy(ob[:, 512:1024], pj[:, 1, :])
            nc.sync.dma_start(out_d[tt * 128:(tt + 1) * 128, :], ob)

        # ---- streamed emission ----
        for tt in range(4):
            emit_v(tt)
        for hp in range(2):
            emit_qk(hp, wq_sb, qT, 0)
            emit_qk(hp, wk_sb, kT, 0)

        def drip_for(tb):
            """(deadline_si, fn) units to drain during tb's si loop."""
            work = []
            if tb + 1 < TB:
                # q/k of the next block: needed early by the QK run-ahead
                for i, (hp, w_sb, dst) in enumerate(
                        ((0, wq_sb, qT), (0, wk_sb, kT),
                         (1, wq_sb, qT), (1, wk_sb, kT))):
                    work.append((i // 2,
                                 lambda hp=hp, w=w_sb, d=dst, tb=tb + 1:
                                 emit_qk(hp, w, d, tb)))
            if tb >= 1:
                # v tiles of this block: needed by PV at si == tt
                for ttl in range(4):
                    tt = 4 * tb + ttl
                    work.append((tt - 1, lambda tt=tt: emit_v(tt)))
            work.sort(key=lambda u: u[0])
            return work

        # initial zeroing of the PV regions (start=False chains)
        for ttl in range(4):
            for h0, n, base in region_runs(ttl):
                nc.vector.memset(arena[:, base:base + 65 * n], 0.0)

        # norm lags PV by one si and proj by two, so the PE's next QK is
        # never queued behind the DVE normalize chain. QK+exp of block tb+1
        # runs ahead inside tb's loop (pt slot si frees once PV(tb, si) is
        # emitted), so the exp stream saturates the Activation engine early
        # instead of back-loading the final block.
        pending_norm = None    # (tb, tt)
        pending_proj = None
        ahead = 0              # QKEs of the current tb pre-emitted earlier
        for tb in range(TB):
            work = drip_for(tb)
            n_si = 4 * tb + 4
            done = 0
            ahead, next_ahead = 0 if tb == 0 else ahead, 0
            for si in range(n_si):
                if si >= ahead:
                    for hp in range(2):
                        emit_qk_si(hp, tb, si)
                if pending_proj is not None:
                    emit_proj(pending_proj)
                    pending_proj = None
                if pending_norm is not None:
                    emit_norm(*pending_norm)
                    pending_proj = pending_norm[1]
                    pending_norm = None
                # deadline-due units first, then even-rate filler
                want = max(done, (si + 1) * len(work) // n_si)
                while done < len(work) and (work[done][0] <= si or
                                            done < want):
                    work[done][1]()
                    done += 1
                emit_pv_si(tb, si)
                if si >= 4 * tb:
                    pending_norm = (tb, si)
                if tb + 1 < TB:
                    while next_ahead < si:
                        for hp in range(2):
                            emit_qk_si(hp, tb + 1, next_ahead)
                        next_ahead += 1
            ahead = next_ahead
        emit_proj(pending_proj)
        emit_norm(*pending_norm)
        emit_proj(pending_norm[1])


_NC_CACHE = None


def _get_nc():
    global _NC_CACHE
    if _NC_CACHE is None:
        _NC_CACHE = build()
    return _NC_CACHE


def _in_maps(x, W_attn, W_proj):
    x16 = x.astype(np.float16)
    wa16 = W_attn.astype(np.float16)
    wp16 = W_proj.astype(np.float16)
    maps = []
    for core in range(N_CORES):
        b, g = core // 4, core % 4
        f0 = FC * g
        maps.append({
            "x": np.ascontiguousarray(x16[b]),
            "wq": np.ascontiguousarray(wa16[:, f0:f0 + FC]),
            "wk": np.ascontiguousarray(wa16[:, C + f0:C + f0 + FC]),
            "wv": np.ascontiguousarray(wa16[:, 2 * C + f0:2 * C + f0 + FC]),
            "wp": np.ascontiguousarray(wp16[f0:f0 + FC, :]),
        })
    return maps


def run(x, W_attn, W_proj, trace=False, **kwargs):
    nc = _get_nc()
    res = run_bass_kernel_spmd(nc, _in_maps(x, W_attn, W_proj),
                               core_ids=list(range(N_CORES)),
                               trace=trace, **kwargs)
    out = np.zeros((B, T, C), dtype=np.float32)
    for core in range(N_CORES):
        out[core // 4] += res.results[core]["out"]
    return out, res


def kernel(x, W_attn, W_proj):
    x = np.asarray(x, dtype=np.float32)
    W_attn = np.asarray(W_attn, dtype=np.float32)
    W_proj = np.asarray(W_proj, dtype=np.float32)
    out, _ = run(x, W_attn, W_proj, trace=False)
    return out


# revision 20
# speedup vs baseline: 1.2017x; 1.2017x over previous
"""Causal self-attention Trainium2 kernel (B=2, T=2048, C=1024, H=16).

Sharding: tensor-parallel over heads (4-way) x data-parallel over batch (2-way)
= 8 cores. Core c handles batch b = c//4 and heads [4*(c%4), 4*(c%4)+4).
Each core computes x @ W_attn for its head slice, causal attention for its 4
heads, and a partial y @ W_proj over its 256 channels. The host sums the 4
partials per batch element (no device collectives).

Matmul operands are fp16 (full-rate PE; fp32 matmul is 4x slower). All PSUM
accumulation is fp32. Weights are host-cast to fp16.

Layouts (per core, b fixed):
  xT   [c, t]    : 8 c-tiles of [128, 2048]  (DMA-xbar transposed from x)
  qT/kT[d', t]   : per head-pair hp, [128, 2048]; partitions 0-63 = head 2hp,
                   64-127 = head 2hp+1
  vp   [s, h, d']: [128, 16 s-tiles, 4 heads, 65]; col 64 is a ones-column so
                   PV emits the softmax denominator for free
  sp   [s, hi, t]: scores for one s-tile, both heads of pair hp, PSUM
                   [128,2,512]; one exp covers both heads, causally trimmed
  y2   [t, h, 65]: PV output t-major in a 3-bank PSUM arena; col 64 is the
                   denominator. Normalize is a per-partition tensor_scalar
                   multiply, then a PE transpose back to [c', t] for proj.

Emission is si-outer: per s-tile, QK -> exp -> PV matmuls for every t-tile
at or above the diagonal, so the PE has PV work during the exp-bound early
s-tiles and the per-block tail is tiny. qkv matmuls for later t-blocks are
drip-fed into the stream with per-unit deadlines.
"""

import sys

if "/opt/trn_rl_repo" not in sys.path:
    sys.path.insert(0, "/opt/trn_rl_repo")

import numpy as np

import concourse.bass as bass
import concourse.bacc as bacc
import concourse.mybir as mybir
import concourse.tile as tile
from concourse.bass_utils import run_bass_kernel_spmd

F32 = mybir.dt.float32
F16 = mybir.dt.float16

B, T, C = 2, 2048, 1024
NH = 16              # total heads
D = 64               # head dim
N_CORES = 8
HG = 4               # heads per core
FC = HG * D          # 256 f-columns per core per q/k/v
CT = C // 128        # 8 c-tiles
TT = T // 128        # 16 t-tiles / s-tiles
TB = T // 512        # 4 t-blocks
SCALE = 1.0 / 8.0    # 1/sqrt(D)


def build():
    nc = bacc.Bacc("TRN2", target_bir_lowering=False, debug=False,
                   num_devices=N_CORES)
    x_d = nc.dram_tensor("x", [T, C], F16, kind="ExternalInput").ap()
    wq_d = nc.dram_tensor("wq", [C, FC], F16, kind="ExternalInput").ap()
    wk_d = nc.dram_tensor("wk", [C, FC], F16, kind="ExternalInput").ap()
    wv_d = nc.dram_tensor("wv", [C, FC], F16, kind="ExternalInput").ap()
    wp_d = nc.dram_tensor("wp", [FC, C], F16, kind="ExternalInput").ap()
    out_d = nc.dram_tensor("out", [T, C], F32, kind="ExternalOutput").ap()

    with tile.TileContext(nc) as tc:
        body(tc, x_d, wq_d, wk_d, wv_d, wp_d, out_d)
    nc.compile()
    return nc


def body(tc, x_d, wq_d, wk_d, wv_d, wp_d, out_d):
    nc = tc.nc
    Exp = mybir.ActivationFunctionType.Exp

    with (
        tc.tile_pool(name="sb", bufs=1) as sb,
        tc.tile_pool(name="ps", bufs=1, space="PSUM") as ps,
    ):
        # binary causal mask in S^T orientation: 1 where t - s >= 0 else 0
        mask = sb.tile([128, 128], F16)
        nc.gpsimd.memset(mask, 1.0)
        nc.gpsimd.affine_select(
            out=mask, in_=mask, compare_op=mybir.AluOpType.is_ge,
            fill=0.0, base=0, pattern=[[1, 128]], channel_multiplier=-1)
        # identity for PE transposes: intersect the two triangle selects
        ident = sb.tile([128, 128], F16)
        nc.gpsimd.memset(ident, 1.0)
        nc.gpsimd.affine_select(
            out=ident, in_=ident, compare_op=mybir.AluOpType.is_ge,
            fill=0.0, base=0, pattern=[[1, 128]], channel_multiplier=-1)
        nc.gpsimd.affine_select(
            out=ident, in_=ident, compare_op=mybir.AluOpType.is_ge,
            fill=0.0, base=0, pattern=[[-1, 128]], channel_multiplier=1)
        ones = sb.tile([128, 64], F16)
        nc.gpsimd.memset(ones, 1.0)

        wq_sb = sb.tile([128, CT, FC], F16)
        wk_sb = sb.tile([128, CT, FC], F16)
        wv_sb = sb.tile([128, CT, FC], F16)
        wp_sb = sb.tile([128, 2, C], F16)          # [c'(128), hp, n]
        nc.gpsimd.dma_start(wv_sb, wv_d.rearrange("(ct p) f -> p ct f", p=128))
        xT = sb.tile([128, CT, T], F16)            # [c_local, ct, t]
        qT = sb.tile([128, 2, T], F16)             # [d', hp, t]
        kT = sb.tile([128, 2, T], F16)
        vp = sb.tile([128, TT, HG, 65], F16)       # [s_in_tile, s_tile, h, d'+1]
        pt0 = sb.tile([128, TT, 2, 512], F16)      # [s, s_tile, hi, t_in_tb]
        pt1 = sb.tile([128, TT, 2, 512], F16)
        pts = (pt0, pt1)
        yT = sb.tile([128, 2, T], F16)             # [c', hp, t]
        nc.gpsimd.memset(vp[:, :, :, 64:65], 1.0)  # PV denominator column

        # transpose x into xT with the DMA xbar (fp16, 2-byte dtype),
        # one [512, 128] -> [128, 512] transpose per (t-block, c-tile).
        # weight loads go out on the Pool engine's DMA queue so SP's slow
        # per-instruction issue (~860ns) is spent only on the x transposes,
        # which gate the compute start
        nc.gpsimd.dma_start(wq_sb, wq_d.rearrange("(ct p) f -> p ct f", p=128))
        nc.gpsimd.dma_start(wk_sb, wk_d.rearrange("(ct p) f -> p ct f", p=128))
        nc.gpsimd.dma_start(wp_sb, wp_d.rearrange("(hp p) n -> p hp n", p=128))
        # tb1's transposes ride the Activation DMA queue concurrently with
        # SP's tb0 stream: the qk(tb1) drip needs them by ~10us
        for tb in (0, 1, 2, 3):
            eng = nc.scalar if tb == 1 else nc.sync
            for ci in range(CT):
                eng.dma_start(
                    xT[:, ci, tb * 512:(tb + 1) * 512],
                    x_d[tb * 512:(tb + 1) * 512, ci * 128:(ci + 1) * 128],
                    transpose=True)

        # PSUM: sp 2x2 banks + qv 1 + arena 3 = 8 banks
        def sp_tile(name):
            return ps.tile([128, 2, 512], F32, tag="sp", name=name, bufs=2)

        def qv_tile(name):
            return ps.tile([128, 512], F32, tag="qv", name=name, bufs=1)

        # 3-bank arena: 16 PV accumulation regions of 65 fp32 columns
        # (region r = ttl*4+h at bank r//7, slot r%7). PSUM start_tensor_calc
        # zeroes a whole 2KB bank, which would wipe sibling regions, so the
        # arena is zeroed by DVE memset per t-block and every PV matmul
        # accumulates with start=False.
        arena = ps.tile([128, 1536], F32, name="arena")

        def reg(h, ttl):
            r = ttl * 4 + h
            bank, idx = divmod(r, 7)
            c0 = bank * 512 + idx * 65
            return arena[:, c0:c0 + 65]

        # HAM warmup: the PE is DMA-blocked at startup; dependency-free
        # matmuls on constants un-throttle the clock, and one tiny exp
        # prefetches the spline table.
        for _ in range(16):
            warm = qv_tile("warm")
            nc.tensor.matmul(warm[0:64, 0:128], lhsT=ones, rhs=mask,
                             start=True, stop=True)
            nc.tensor.matmul(warm[0:64, 128:256], lhsT=ones, rhs=mask,
                             start=True, stop=True)
        warm_e = sb.tile([1, 64], F16)
        nc.scalar.activation(warm_e, ones[0:1, :], Exp, scale=SCALE)

        def emit_v(tt):
            v_ps = qv_tile("v_ps")
            for ci in range(CT):
                nc.tensor.matmul(
                    v_ps[:, 0:256],
                    lhsT=xT[:, ci, tt * 128:(tt + 1) * 128],
                    rhs=wv_sb[:, ci, :],
                    start=(ci == 0), stop=(ci == CT - 1))
            nc.vector.tensor_copy(
                vp[:, tt, :, 0:64],
                v_ps[:, 0:256].rearrange("p (h d) -> p h d", h=HG))

        def emit_qk(hp, w_sb, dst, tb):
            qk_ps = qv_tile("qk_ps")
            for ci in range(CT):
                nc.tensor.matmul(
                    qk_ps,
                    lhsT=w_sb[:, ci, hp * 128:(hp + 1) * 128],
                    rhs=xT[:, ci, tb * 512:(tb + 1) * 512],
                    start=(ci == 0), stop=(ci == CT - 1))
            nc.vector.tensor_copy(
                dst[:, hp, tb * 512:(tb + 1) * 512], qk_ps)

        def emit_qk_si(hp, tb, si):
            # scores S^T for one s-tile, both heads of pair hp, then one
            # exp covering both heads with the causal prefix trimmed
            kd = si - 4 * tb
            col0 = 128 * kd if kd > 0 else 0
            sp = sp_tile("sp")
            for hi in (0, 1):
                nc.tensor.matmul(
                    sp[:, hi, col0:512],
                    lhsT=kT[64 * hi:64 * hi + 64, hp,
                            si * 128:(si + 1) * 128],
                    rhs=qT[64 * hi:64 * hi + 64, hp,
                           tb * 512 + col0:(tb + 1) * 512],
                    start=True, stop=True)
            pt = pts[hp]
            nc.scalar.activation(pt[:, si, :, col0:512], sp[:, :, col0:512],
                                 Exp, scale=SCALE)
            if kd >= 0:
                # zero the invalid triangle of the diagonal square after
                # exp (exp * 0 == masked exp, off the S -> exp hot path)
                for hi in (0, 1):
                    psl = pt[:, si, hi, col0:col0 + 128]
                    nc.vector.tensor_mul(psl, psl, mask)

        def emit_pv_si(tb, si):
            # PV matmuls of s-tile si into every t-tile at/above the
            # diagonal; chain (h, ttl) accumulates over si and closes at
            # the diagonal
            for ttl in range(max(0, si - 4 * tb), 4):
                tt = 4 * tb + ttl
                for h in range(HG):
                    hp, hi = h // 2, h % 2
                    nc.tensor.matmul(
                        reg(h, ttl),
                        lhsT=pts[hp][:, si, hi, ttl * 128:(ttl + 1) * 128],
                        rhs=vp[:, si, h, :],
                        start=False, stop=(si == tt),
                        skip_group_check=True)

        def region_runs(ttl):
            """maximal same-bank runs of the 4 regions of t-tile ttl"""
            runs = []
            h0 = 0
            while h0 < HG:
                r0 = ttl * 4 + h0
                n = min(HG - h0, 7 - r0 % 7)
                bank, idx = divmod(r0, 7)
                runs.append((h0, n, bank * 512 + idx * 65))
                h0 += n
            return runs

        def emit_norm(tb, tt):
            ttl = tt - 4 * tb
            runs = region_runs(ttl)
            rcp = sb.tile([128, HG], F32, tag="rcp", name="rcp", bufs=3)
            for h0, n, base in runs:
                sl = arena[:, base:base + 65 * n].rearrange(
                    "p (n x) -> p n x", n=n)
                nc.vector.reciprocal(rcp[:, h0:h0 + n], sl[:, :, 64])
            y_sb = sb.tile([128, FC], F16, tag="ysb", name="y_sb", bufs=3)
            for h0, n, base in runs:
                sl = arena[:, base:base + 65 * n].rearrange(
                    "p (n x) -> p n x", n=n)
                nc.vector.tensor_mul(
                    y_sb[:, h0 * 64:(h0 + n) * 64].rearrange(
                        "p (n x) -> p n x", n=n),
                    sl[:, :, 0:64],
                    rcp[:, h0:h0 + n].unsqueeze(2).broadcast_to([128, n, 64]))
            # PE transpose [t, c'] -> [c', t] for the projection lhsT; the
            # scratch borrows an sp rotation slot (start=True zeroing is
            # safe there, unlike in the accumulating arena)
            yt = sp_tile("yt")[:, 0, 0:128].bitcast(F16)   # [128, 256] f16
            for ch in range(2):
                nc.tensor.transpose(
                    yt[:, ch * 128:(ch + 1) * 128],
                    y_sb[:, ch * 128:(ch + 1) * 128], ident)
            nc.vector.tensor_copy(
                yT[:, :, tt * 128:(tt + 1) * 128],
                yt.rearrange("p (hp t) -> p hp t", hp=2))
            # re-zero this t-tile's regions for the next block's start=False
            # accumulation chains (WAR-ordered after the reads above)
            for h0, n, base in region_runs(ttl):
                nc.vector.memset(arena[:, base:base + 65 * n], 0.0)

        def emit_proj(tt):
            pj = sp_tile("pj")
            for nb in range(2):
                for hp in range(2):
                    nc.tensor.matmul(
                        pj[:, nb, :],
                        lhsT=yT[:, hp, tt * 128:(tt + 1) * 128],
                        rhs=wp_sb[:, hp, nb * 512:(nb + 1) * 512],
                        start=(hp == 0), stop=(hp == 1))
            ob = sb.tile([128, C], F32, tag="ob", name="ob", bufs=3)
            nc.scalar.copy(ob[:, 0:512], pj[:, 0, :])
            nc.vector.tensor_copy(ob[:, 512:1024], pj[:, 1, :])
            nc.sync.dma_start(out_d[tt * 128:(tt + 1) * 128, :], ob)

        # ---- streamed emission ----
        for tt in range(4):
            emit_v(tt)
        for hp in range(2):
            emit_qk(hp, wq_sb, qT, 0)
            emit_qk(hp, wk_sb, kT, 0)

        def drip_for(tb):
            """(deadline_si, fn) units to drain during tb's si loop."""
            work = []
            if tb + 1 < TB:
                # q/k of the next block: needed early by the QK run-ahead
                for i, (hp, w_sb, dst) in enumerate(
                        ((0, wq_sb, qT), (0, wk_sb, kT),
                         (1, wq_sb, qT), (1, wk_sb, kT))):
                    work.append((i // 2,
                                 lambda hp=hp, w=w_sb, d=dst, tb=tb + 1:
                                 emit_qk(hp, w, d, tb)))
            if tb >= 1:
                # v tiles of this block: needed by PV at si == tt
                for ttl in range(4):
                    tt = 4 * tb + ttl
                    work.append((tt - 1, lambda tt=tt: emit_v(tt)))
            work.sort(key=lambda u: u[0])
            return work

        # initial zeroing of the PV regions (start=False chains)
        for ttl in range(4):
            for h0, n, base in region_runs(ttl):
                nc.vector.memset(arena[:, base:base + 65 * n], 0.0)

        # norm lags PV by one si and proj by two, so the PE's next QK is
        # never queued behind the DVE normalize chain. QK+exp of block tb+1
        # runs ahead inside tb's loop (pt slot si frees once PV(tb, si) is
        # emitted), so the exp stream saturates the Activation engine early
        # instead of back-loading the final block.
        pending_norm = None    # (tb, tt)
        pending_proj = None
        ahead = 0              # QKEs of the current tb pre-emitted earlier
        for tb in range(TB):
            work = drip_for(tb)
            n_si = 4 * tb + 4
            done = 0
            ahead, next_ahead = 0 if tb == 0 else ahead, 0
            for si in range(n_si):
                if si >= ahead:
                    for hp in range(2):
                        emit_qk_si(hp, tb, si)
                if pending_proj is not None:
                    emit_proj(pending_proj)
                    pending_proj = None
                if pending_norm is not None:
                    emit_norm(*pending_norm)
                    pending_proj = pending_norm[1]
                    pending_norm = None
                # deadline-due units first, then even-rate filler
                want = max(done, (si + 1) * len(work) // n_si)
                while done < len(work) and (work[done][0] <= si or
                                            done < want):
                    work[done][1]()
                    done += 1
                emit_pv_si(tb, si)
                if si >= 4 * tb:
                    pending_norm = (tb, si)
                if tb + 1 < TB:
                    while next_ahead < si:
                        for hp in range(2):
                            emit_qk_si(hp, tb + 1, next_ahead)
                        next_ahead += 1
            ahead = next_ahead
        emit_proj(pending_proj)
        emit_norm(*pending_norm)
        emit_proj(pending_norm[1])


_NC_CACHE = None


def _get_nc():
    global _NC_CACHE
    if _NC_CACHE is None:
        _NC_CACHE = build()
    return _NC_CACHE


def _in_maps(x, W_attn, W_proj):
    x16 = x.astype(np.float16)
    wa16 = W_attn.astype(np.float16)
    wp16 = W_proj.astype(np.float16)
    maps = []
    for core in range(N_CORES):
        b, g = core // 4, core % 4
        f0 = FC * g
        maps.append({
            "x": np.ascontiguousarray(x16[b]),
            "wq": np.ascontiguousarray(wa16[:, f0:f0 + FC]),
            "wk": np.ascontiguousarray(wa16[:, C + f0:C + f0 + FC]),
            "wv": np.ascontiguousarray(wa16[:, 2 * C + f0:2 * C + f0 + FC]),
            "wp": np.ascontiguousarray(wp16[f0:f0 + FC, :]),
        })
    return maps


def run(x, W_attn, W_proj, trace=False, **kwargs):
    nc = _get_nc()
    res = run_bass_kernel_spmd(nc, _in_maps(x, W_attn, W_proj),
                               core_ids=list(range(N_CORES)),
                               trace=trace, **kwargs)
    out = np.zeros((B, T, C), dtype=np.float32)
    for core in range(N_CORES):
        out[core // 4] += res.results[core]["out"]
    return out, res


def kernel(x, W_attn, W_proj):
    x = np.asarray(x, dtype=np.float32)
    W_attn = np.asarray(W_attn, dtype=np.float32)
    W_proj = np.asarray(W_proj, dtype=np.float32)
    out, _ = run(x, W_attn, W_proj, trace=False)
    return out
